# revision 8
# baseline (speedup 1.0000x reference)
"""DeepAR (2-layer LSTM, H=512) Trainium2 Bass kernel.

Full-input contract: kernel(**inputs) takes the unsharded inputs from
setup_inputs() and returns the full [512, 64, 2] output.  Internally the
batch (512) is sharded 64-per-core across 8 NeuronCores (data parallel);
LSTM weights are replicated.

Device strategy (per core, B=64):
  - Weights resident in SBUF, streamed through the PE as the MOVING
    matmul operand (float32r, N=512 -> ~150ns/MM incl. weight load).
  - Gate columns are host-permuted into per-hidden-half blocks
    [GOa | IFa | GOb | IFb] (a = hidden 0:256, b = 256:512), so each
    N=512 matmul fills one [64, 512] PSUM tile that is a self-contained
    (g|o) or (i|f) pair for one half.  The LSTM cell update then runs
    as two half-width pipelines: tanh(g_h) lands in a composite tile
    [g_h | c_h], one DVE mul gives (i*g | f*c), one add updates c_h.
  - L1 bias (and +1 forget bias) folds in via the ones-row of the xy
    feature chunk; L2 bias is injected as K=1 ones x b2-row matmuls
    that open each PSUM accumulation group (no vector-engine cost).
  - h halves transpose back to [H, B] chunks via PE transpose as soon
    as each half is ready; transpose/head PSUM shares banks with gate
    tiles whose reads retire early (all 8 banks are gate storage).
  - Autoregressive decode feeds m = h2 @ Wm + bm back into the feature
    row in-place in SBUF; mean/disp outputs accumulate in the same tile.
"""
import sys

sys.path.insert(0, "/opt/trn_rl_repo")

import numpy as np

import concourse.bass as bass
import concourse.mybir as mybir
from concourse import bass_utils, tile

F32 = mybir.dt.float32
F32R = mybir.dt.float32r
Act = mybir.ActivationFunctionType

B_FULL, TP, TO, F, H = 512, 192, 128, 64, 512
NC = 8
B = B_FULL // NC            # 64 per core
G = 4 * H                   # 2048 gate width
HH = H // 2                 # 256 half-hidden
NSLOT = TP + 1              # 193 feature slots (slot t feeds step t)
XCOLS = NSLOT * B           # 12352


def gate_perm():
    """Column permutation of the 2048 gate axis into [GOa|IFa|GOb|IFb].
    haiku order: i=[0:512], g=[512:1024], f=[1024:1536], o=[1536:2048]."""
    r0 = np.arange(HH)
    r1 = HH + np.arange(HH)
    gi, gg, gf, go = 0, H, 2 * H, 3 * H
    return np.concatenate([
        gg + r0, go + r0,      # GOa
        gi + r0, gf + r0,      # IFa
        gg + r1, go + r1,      # GOb
        gi + r1, gf + r1,      # IFb
    ])


def ts(i, n):
    return slice(i * n, (i + 1) * n)


def split_excess_waits(nc):
    """Walrus accepts only one sync-wait per hardware instruction. Hoist
    excess waits onto NoOps (same engine) inserted right before."""
    n = 0
    for f in nc.m.functions:
        for blk in f.blocks:
            out = []
            for inst in blk.instructions:
                si = inst.sync_info
                if si is not None and si.on_wait and len(si.on_wait) > 1:
                    waits = list(si.on_wait)
                    for j, w in enumerate(waits[:-1]):
                        nop = mybir.InstNoOp(
                            name=f"{inst.name}-wnop{j}", ins=[], outs=[])
                        nop.engine = inst.engine
                        nop.sync_info = mybir.SyncInfo(on_wait=[w], on_update=[])
                        out.append(nop)
                        n += 1
                    inst.sync_info = mybir.SyncInfo(
                        on_wait=[waits[-1]], on_update=list(si.on_update))
                out.append(inst)
            blk.instructions = out
    return n


def build_program(tp=TP, to=TO, split_waits=True):
    NSLOT_ = tp + 1
    XCOLS_ = NSLOT_ * B
    nc = bass.Bass("TRN2", target_bir_lowering=False, debug=False,
                   num_devices=NC)

    xyf_d = nc.dram_tensor("xyf_d", [66, XCOLS_], F32R, kind="ExternalInput").ap()
    w1c0_d = nc.dram_tensor("w1c0_d", [66, G], F32R, kind="ExternalInput").ap()
    w1h_d = nc.dram_tensor("w1h_d", [128, 4 * G], F32R, kind="ExternalInput").ap()
    w2_d = nc.dram_tensor("w2_d", [128, 8 * G], F32R, kind="ExternalInput").ap()
    wmd_d = nc.dram_tensor("wmd_d", [128, 4 * 64], F32R, kind="ExternalInput").ap()
    b2row_d = nc.dram_tensor("b2row_d", [1, G], F32R, kind="ExternalInput").ap()
    bmd_d = nc.dram_tensor("bmd_d", [33, 1], F32, kind="ExternalInput").ap()
    id_d = nc.dram_tensor("id_d", [64, 64], F32, kind="ExternalInput").ap()
    out_d = nc.dram_tensor("out_d", [2, (tp - to) * B], F32,
                           kind="ExternalOutput").ap()

    with tile.TileContext(nc) as tc:
        with tc.sbuf_pool(name="const", bufs=1) as cp, \
             tc.sbuf_pool(name="work", bufs=1) as wp, \
             tc.psum_pool(name="ps", bufs=1) as pp:
            # ---- persistent tiles + input DMA ----
            xyf = cp.tile([66, XCOLS_], F32R, name="xyf")
            w1c0 = cp.tile([66, G], F32R, name="w1c0")
            w1h = cp.tile([128, 4 * G], F32R, name="w1h")
            w2 = cp.tile([128, 8 * G], F32R, name="w2")
            wmd = cp.tile([128, 4 * 64], F32R, name="wmd")
            b2row = cp.tile([1, G], F32R, name="b2row")
            bmd = cp.tile([33, 1], F32, name="bmd")
            ident = cp.tile([64, 64], F32, name="ident")

            nc.sync.dma_start(xyf[:, :], xyf_d[:, :])
            nc.sync.dma_start(w1c0[:, :], w1c0_d[:, :])
            for k in range(4):
                nc.sync.dma_start(w1h[:, ts(k, G)], w1h_d[:, ts(k, G)])
            for k in range(8):
                nc.sync.dma_start(w2[:, ts(k, G)], w2_d[:, ts(k, G)])
            nc.sync.dma_start(wmd[:, :], wmd_d[:, :])
            nc.sync.dma_start(b2row[:, :], b2row_d[:, :])
            nc.sync.dma_start(bmd[:, :], bmd_d[:, :])
            nc.sync.dma_start(ident[:, :], id_d[:, :])

            # ---- state composites: CC = [g_h | c_h] per layer per half ----
            CC1 = [cp.tile([64, 2 * HH], F32, name=f"CC1{hf}")
                   for hf in "ab"]
            CC2 = [cp.tile([64, 2 * HH], F32, name=f"CC2{hf}")
                   for hf in "ab"]
            for t_ in CC1 + CC2:
                nc.vector.memset(t_[:, :], 0.0)

            ones_t = cp.tile([1, 64], F32, name="ones_t")
            nc.vector.memset(ones_t[:, :], 1.0)
            ONES = ones_t[:, :].bitcast(F32R)

            # per-step psum: 8 gate tiles x 1 bank (tags reused for trp/mdp)
            def gate_tiles(layer):
                tags = [f"go_a{layer}", f"if_a{layer}",
                        f"go_b{layer}", f"if_b{layer}"]
                return [pp.tile([64, 2 * HH], F32, name=t, tag=t, bufs=1)
                        for t in tags]

            def half_post(ps_go, ps_if, CC, htag, h_half):
                """one half: psums -> h_half [64, 256] sbuf write."""
                nc.scalar.activation(CC[:, 0:HH], ps_go[:, 0:HH], Act.Tanh)
                sx = wp.tile([64, 2 * HH], F32, name=f"sx{htag}",
                             tag=f"sx{htag}")
                nc.scalar.activation(sx[:, :], ps_if[:, :], Act.Sigmoid)
                m_t = wp.tile([64, 2 * HH], F32, name=f"m{htag}",
                              tag=f"m{htag}")
                nc.vector.tensor_mul(m_t[:, :], sx[:, :], CC[:, :])
                nc.vector.tensor_add(CC[:, HH:2 * HH], m_t[:, 0:HH],
                                     m_t[:, HH:2 * HH])
                os_t = wp.tile([64, HH], F32, name=f"os{htag}",
                               tag=f"os{htag}")
                nc.scalar.activation(os_t[:, :], ps_go[:, HH:2 * HH],
                                     Act.Sigmoid)
                tc_s = wp.tile([64, HH], F32, name=f"tc{htag}",
                               tag=f"tc{htag}")
                nc.scalar.activation(tc_s[:, :], CC[:, HH:2 * HH], Act.Tanh)
                nc.vector.tensor_mul(h_half[:, :], os_t[:, :], tc_s[:, :])

            def transpose_half(h_half, trp, base_kk, htag):
                """h half [64,256] -> 2 hT chunk tiles [128,64]."""
                chunks = []
                for kk in range(2):
                    nc.tensor.transpose(trp[:, ts(kk, 64)],
                                        h_half[:, ts(kk, 128)], ident[:, :])
                for kk in range(2):
                    hTk = wp.tile([128, 64], F32R,
                                  name=f"hT{htag}k{base_kk + kk}",
                                  tag=f"hT{htag}k{base_kk + kk}", bufs=2)
                    nc.vector.tensor_copy(hTk[:, :], trp[:, ts(kk, 64)])
                    chunks.append(hTk)
                return chunks

            h1T = None
            h2T_prev = None
            ps1 = None      # [goa, ifa, gob, ifb] accumulating L1 gates for t

            for t in range(tp):
                first = t == 0
                # --- L2 groups open with bias matmuls (pure PE fill) ---
                ps2 = gate_tiles(2)
                for j in range(4):
                    nc.tensor.matmul(ps2[j][:, :], ONES, b2row[:, ts(j, 512)],
                                     start=True, stop=False,
                                     skip_group_check=True)
                # --- L2 h2-part (needs h2T(t-1)) ---
                if not first:
                    for k in range(4):
                        wk = (4 + k) * G
                        st = h2T_prev[k][:, :]
                        for j in range(4):
                            nc.tensor.matmul(
                                ps2[j][:, :], st,
                                w2[:, wk + j * 512:wk + (j + 1) * 512],
                                start=False, stop=False,
                                skip_group_check=True)
                # --- L1(t): finish gates with the xy chunk ---
                if ps1 is None:
                    ps1 = gate_tiles(1)
                xs = xyf[0:66, ts(t, 64)]
                for j in range(4):
                    nc.tensor.matmul(ps1[j][:, :], xs, w1c0[:, ts(j, 512)],
                                     start=first, stop=True,
                                     skip_group_check=True)
                # --- L1 post, half-pipelined with transposes ---
                h1a = wp.tile([64, HH], F32, name="h1a", tag="h1a")
                h1b = wp.tile([64, HH], F32, name="h1b", tag="h1b")
                half_post(ps1[0], ps1[1], CC1[0], "1a", h1a)
                half_post(ps1[2], ps1[3], CC1[1], "1b", h1b)
                trp1a = pp.tile([128, 128], F32, name="trp1a",
                                tag="go_a1", bufs=1)
                c01 = transpose_half(h1a, trp1a, 0, "1")
                # --- L2 h1-part k=0 between the half transposes ---
                def l2h1_chunk(k, st):
                    wk = k * G
                    for j in range(4):
                        nc.tensor.matmul(
                            ps2[j][:, :], st,
                            w2[:, wk + j * 512:wk + (j + 1) * 512],
                            start=False, stop=(k == 3),
                            skip_group_check=True)
                l2h1_chunk(0, c01[0][:, :])
                trp1b = pp.tile([128, 128], F32, name="trp1b",
                                tag="go_b1", bufs=1)
                c23 = transpose_half(h1b, trp1b, 2, "1")
                h1T = c01 + c23
                for k in range(1, 4):
                    l2h1_chunk(k, h1T[k][:, :])
                # --- L1(t+1) h-part (pipelined ahead) ---
                if t < tp - 1:
                    nps1 = gate_tiles(1)
                    for k in range(4):
                        wk = k * G
                        st = h1T[k][:, :]
                        for j in range(4):
                            nc.tensor.matmul(
                                nps1[j][:, :], st,
                                w1h[:, wk + j * 512:wk + (j + 1) * 512],
                                start=(k == 0), stop=False,
                                skip_group_check=True)
                    ps1 = nps1
                else:
                    ps1 = None
                # --- L2 post + h2 transposes + head ---
                h2a = wp.tile([64, HH], F32, name="h2a", tag="h2a")
                h2b = wp.tile([64, HH], F32, name="h2b", tag="h2b")
                half_post(ps2[0], ps2[1], CC2[0], "2a", h2a)
                half_post(ps2[2], ps2[3], CC2[1], "2b", h2b)
                trp2a = pp.tile([128, 128], F32, name="trp2a",
                                tag="go_a2", bufs=1)
                d01 = transpose_half(h2a, trp2a, 0, "2")
                do_head = t >= to - 1
                if do_head:
                    mdp = pp.tile([64, 64], F32, name="mdp", tag="if_b2",
                                  bufs=1)
                    for k in range(2):
                        nc.tensor.matmul(mdp[:, :], wmd[:, ts(k, 64)],
                                         d01[k][:, :], start=(k == 0),
                                         stop=False, skip_group_check=True)
                trp2b = pp.tile([128, 128], F32, name="trp2b",
                                tag="go_b2", bufs=1)
                d23 = transpose_half(h2b, trp2b, 2, "2")
                h2T = d01 + d23
                if do_head:
                    for k in range(2, 4):
                        nc.tensor.matmul(mdp[:, :], wmd[:, ts(k, 64)],
                                         h2T[k][:, :], start=False,
                                         stop=(k == 3),
                                         skip_group_check=True)
                    # m -> feature row 0, slot t+1 (f32r rounding on write)
                    nc.scalar.activation(xyf[0:1, ts(t + 1, 64)],
                                         mdp[0:1, :], Act.Identity,
                                         bias=bmd[0:1, 0:1], scale=1.0)
                    if t >= to:
                        # d -> row 64 (ones/d row), slot t (already consumed)
                        nc.scalar.activation(xyf[64:65, ts(t, 64)],
                                             mdp[32:33, :], Act.Identity,
                                             bias=bmd[32:33, 0:1], scale=1.0)
                h2T_prev = h2T

            # ---- outputs: mean row = slots TO+1..TP, disp row = slots TO..TP-1
            nc.sync.dma_start(out_d[0:1, :],
                              xyf[0:1, (to + 1) * B:(tp + 1) * B].bitcast(F32))
            nc.sync.dma_start(out_d[1:2, :],
                              xyf[64:65, to * B:tp * B].bitcast(F32))

    n = split_excess_waits(nc) if split_waits else 0
    return nc, n


_CACHE = {}


def _get_program():
    if "nc" not in _CACHE:
        _CACHE["nc"] = build_program()[0]
    return _CACHE["nc"]


def make_core_inputs(x, y, W1, b1, W2, b2, Wm, bm, Wd, bd, tp=TP, to=TO):
    """Host-side prep: returns (in_maps list of 8 dicts, scale [512])."""
    NSLOT_ = tp + 1
    XCOLS_ = NSLOT_ * B
    x = np.asarray(x, np.float32)
    y = np.asarray(y, np.float32)
    W1 = np.asarray(W1, np.float32)
    b1 = np.asarray(b1, np.float32)
    W2 = np.asarray(W2, np.float32)
    b2 = np.asarray(b2, np.float32)
    Wm = np.asarray(Wm, np.float32)
    bm = np.asarray(bm, np.float32)
    Wd = np.asarray(Wd, np.float32)
    bd = np.asarray(bd, np.float32)

    scale = 1.0 + np.mean(y[:, 0:to, 0], axis=1)       # [512]
    y_sc = y[:, 0:to, 0] / scale[:, None]              # [512, to]

    b1a = b1.copy()
    b1a[2 * H:3 * H] += 1.0                             # forget-gate +1
    b2a = b2.copy()
    b2a[2 * H:3 * H] += 1.0

    P = gate_perm()
    W1p = W1[:, P]
    W2p = W2[:, P]
    b1p = b1a[P]
    b2p = b2a[P]

    # row layout: 0 = y/m, 1:64 = x[0:63], 64 = ones/bias (disp storage),
    # 65 = x[63]  (rows 0 and 64 must sit at legal engine partition bases)
    w1c0 = np.empty((66, G), np.float32)
    w1c0[0] = W1p[F]                                    # y/m weight row
    w1c0[1:64] = W1p[0:F - 1]                           # x weight rows 0..62
    w1c0[64] = b1p                                      # bias row (ones input)
    w1c0[65] = W1p[F - 1]                               # x weight row 63

    w1h = np.ascontiguousarray(
        W1p[F + 1:].reshape(4, 128, G).transpose(1, 0, 2).reshape(128, 4 * G))
    w2 = np.ascontiguousarray(
        W2p.reshape(8, 128, G).transpose(1, 0, 2).reshape(128, 8 * G))

    wmd = np.zeros((128, 4, 64), np.float32)
    wmd[:, :, 0] = Wm[:, 0].reshape(4, 128).T
    wmd[:, :, 32] = Wd[:, 0].reshape(4, 128).T
    wmd = np.ascontiguousarray(wmd.reshape(128, 4 * 64))

    b2row = np.ascontiguousarray(b2p.reshape(1, G))
    bmd = np.zeros((33, 1), np.float32)
    bmd[0, 0] = bm[0]
    bmd[32, 0] = bd[0]
    identity = np.eye(64, dtype=np.float32)

    in_maps = []
    for c in range(NC):
        bs = slice(c * B, (c + 1) * B)
        xyf = np.zeros((66, NSLOT_, B), np.float32)
        xyf[0, 1:to, :] = y_sc[bs, 0:to - 1].T          # shifted y feed
        xt = x[bs].transpose(2, 1, 0)                   # [f, t, b]
        xyf[1:64, 0:tp, :] = xt[0:F - 1, 0:tp]          # x rows 0..62
        xyf[65, 0:tp, :] = xt[F - 1, 0:tp]              # x row 63
        xyf[64, :, :] = 1.0                             # ones / bias row
        in_maps.append({
            "xyf_d": np.ascontiguousarray(xyf.reshape(66, XCOLS_)),
            "w1c0_d": w1c0, "w1h_d": w1h, "w2_d": w2, "wmd_d": wmd,
            "b2row_d": b2row, "bmd_d": bmd, "id_d": identity,
        })
    return in_maps, scale


def postprocess(results, scale, tp=TP, to=TO):
    """results: list of 8 dicts with out_d [2, (tp-to)*64] -> [512, tp-to, 2]."""
    out = np.empty((B_FULL, tp - to, 2), np.float32)
    for c in range(NC):
        r = results[c]["out_d"]
        mean_tb = r[0].reshape(tp - to, B)              # [t, b]
        dpre_tb = r[1].reshape(tp - to, B)
        bs = slice(c * B, (c + 1) * B)
        sc = scale[bs]
        out[bs, :, 0] = (mean_tb * sc[None, :]).T
        disp = np.logaddexp(dpre_tb, 0.0)               # softplus
        out[bs, :, 1] = (disp * np.sqrt(sc)[None, :]).T
    return out


def kernel(x, y, W1, b1, W2, b2, Wm, bm, Wd, bd):
    in_maps, scale = make_core_inputs(x, y, W1, b1, W2, b2, Wm, bm, Wd, bd)
    nc = _get_program()
    res = bass_utils.run_bass_kernel_spmd(nc, in_maps, core_ids=list(range(NC)))
    return postprocess(res.results, scale)


# revision 9
# speedup vs baseline: 1.2445x; 1.2445x over previous
"""DeepAR (2-layer LSTM, H=512) Trainium2 Bass kernel.

Full-input contract: kernel(**inputs) takes the unsharded inputs from
setup_inputs() and returns the full [512, 64, 2] output.  Internally the
batch (512) is sharded 64-per-core across 8 NeuronCores (data parallel);
LSTM weights are replicated.

Device strategy (per core, B=64):
  - Weights resident in SBUF, streamed through the PE as the MOVING
    matmul operand (float32r, N=512 -> ~150ns/MM incl. weight load).
  - Gate columns are host-permuted into per-hidden-half blocks
    [GOa | IFa | GOb | IFb] (a = hidden 0:256, b = 256:512), so each
    N=512 matmul fills one [64, 512] PSUM tile that is a self-contained
    (g|o) or (i|f) pair for one half.  The LSTM cell update then runs
    as two half-width pipelines: tanh(g_h) lands in a composite tile
    [g_h | c_h], one DVE mul gives (i*g | f*c), one add updates c_h.
  - L1 bias (and +1 forget bias) folds in via the ones-row of the xy
    feature chunk; L2 bias is injected as K=1 ones x b2-row matmuls
    that open each PSUM accumulation group (no vector-engine cost).
  - h halves transpose back to [H, B] chunks via PE transpose as soon
    as each half is ready; transpose/head PSUM shares banks with gate
    tiles whose reads retire early (all 8 banks are gate storage).
  - Autoregressive decode feeds m = h2 @ Wm + bm back into the feature
    row in-place in SBUF; mean/disp outputs accumulate in the same tile.
"""
import sys

sys.path.insert(0, "/opt/trn_rl_repo")

import numpy as np

import concourse.bass as bass
import concourse.mybir as mybir
from concourse import bass_utils, tile

F32 = mybir.dt.float32
F32R = mybir.dt.float32r
Act = mybir.ActivationFunctionType

B_FULL, TP, TO, F, H = 512, 192, 128, 64, 512
NC = 8
B = B_FULL // NC            # 64 per core
G = 4 * H                   # 2048 gate width
HH = H // 2                 # 256 half-hidden
NSLOT = TP + 1              # 193 feature slots (slot t feeds step t)
XCOLS = NSLOT * B           # 12352


def gate_perm():
    """Column permutation of the 2048 gate axis into [GOa|IFa|GOb|IFb].
    haiku order: i=[0:512], g=[512:1024], f=[1024:1536], o=[1536:2048]."""
    r0 = np.arange(HH)
    r1 = HH + np.arange(HH)
    gi, gg, gf, go = 0, H, 2 * H, 3 * H
    return np.concatenate([
        gg + r0, go + r0,      # GOa
        gi + r0, gf + r0,      # IFa
        gg + r1, go + r1,      # GOb
        gi + r1, gf + r1,      # IFb
    ])


def ts(i, n):
    return slice(i * n, (i + 1) * n)


def split_excess_waits(nc):
    """Walrus accepts only one sync-wait per hardware instruction. Hoist
    excess waits onto NoOps (same engine) inserted right before."""
    n = 0
    for f in nc.m.functions:
        for blk in f.blocks:
            out = []
            for inst in blk.instructions:
                si = inst.sync_info
                if si is not None and si.on_wait and len(si.on_wait) > 1:
                    waits = list(si.on_wait)
                    for j, w in enumerate(waits[:-1]):
                        nop = mybir.InstNoOp(
                            name=f"{inst.name}-wnop{j}", ins=[], outs=[])
                        nop.engine = inst.engine
                        nop.sync_info = mybir.SyncInfo(on_wait=[w], on_update=[])
                        out.append(nop)
                        n += 1
                    inst.sync_info = mybir.SyncInfo(
                        on_wait=[waits[-1]], on_update=list(si.on_update))
                out.append(inst)
            blk.instructions = out
    return n


def build_program(tp=TP, to=TO, split_waits=True):
    NSLOT_ = tp + 1
    XCOLS_ = NSLOT_ * B
    nc = bass.Bass("TRN2", target_bir_lowering=False, debug=False,
                   num_devices=NC)

    xyf_d = nc.dram_tensor("xyf_d", [66, XCOLS_], F32R, kind="ExternalInput").ap()
    w1c0_d = nc.dram_tensor("w1c0_d", [66, G], F32R, kind="ExternalInput").ap()
    w1h_d = nc.dram_tensor("w1h_d", [128, 4 * G], F32R, kind="ExternalInput").ap()
    w2_d = nc.dram_tensor("w2_d", [128, 8 * G], F32R, kind="ExternalInput").ap()
    wmd_d = nc.dram_tensor("wmd_d", [128, 4 * 64], F32R, kind="ExternalInput").ap()
    b2row_d = nc.dram_tensor("b2row_d", [1, G], F32R, kind="ExternalInput").ap()
    bmd_d = nc.dram_tensor("bmd_d", [33, 1], F32, kind="ExternalInput").ap()
    id_d = nc.dram_tensor("id_d", [64, 64], F32, kind="ExternalInput").ap()
    out_d = nc.dram_tensor("out_d", [2, (tp - to) * B], F32,
                           kind="ExternalOutput").ap()

    with tile.TileContext(nc) as tc:
        with tc.sbuf_pool(name="const", bufs=1) as cp, \
             tc.sbuf_pool(name="work", bufs=1) as wp, \
             tc.psum_pool(name="ps", bufs=1) as pp:
            # ---- persistent tiles + input DMA ----
            xyf = cp.tile([66, XCOLS_], F32R, name="xyf")
            w1c0 = cp.tile([66, G], F32R, name="w1c0")
            w1h = cp.tile([128, 4 * G], F32R, name="w1h")
            w2 = cp.tile([128, 8 * G], F32R, name="w2")
            wmd = cp.tile([128, 4 * 64], F32R, name="wmd")
            b2row = cp.tile([1, G], F32R, name="b2row")
            bmd = cp.tile([33, 1], F32, name="bmd")
            ident = cp.tile([64, 64], F32, name="ident")

            nc.sync.dma_start(xyf[:, :], xyf_d[:, :])
            nc.sync.dma_start(w1c0[:, :], w1c0_d[:, :])
            for k in range(4):
                nc.sync.dma_start(w1h[:, ts(k, G)], w1h_d[:, ts(k, G)])
            for k in range(8):
                nc.sync.dma_start(w2[:, ts(k, G)], w2_d[:, ts(k, G)])
            nc.sync.dma_start(wmd[:, :], wmd_d[:, :])
            nc.sync.dma_start(b2row[:, :], b2row_d[:, :])
            nc.sync.dma_start(bmd[:, :], bmd_d[:, :])
            nc.sync.dma_start(ident[:, :], id_d[:, :])

            # ---- state composites: CC = [g_h | c_h] per layer per half ----
            CC1 = [cp.tile([64, 2 * HH], F32, name=f"CC1{hf}")
                   for hf in "ab"]
            CC2 = [cp.tile([64, 2 * HH], F32, name=f"CC2{hf}")
                   for hf in "ab"]
            for t_ in CC1 + CC2:
                nc.vector.memset(t_[:, :], 0.0)

            ones_t = cp.tile([1, 64], F32, name="ones_t")
            nc.vector.memset(ones_t[:, :], 1.0)
            ONES = ones_t[:, :].bitcast(F32R)

            # per-step psum: 8 gate tiles x 1 bank (tags reused for trp/mdp)
            def gate_tiles(layer):
                tags = [f"go_a{layer}", f"if_a{layer}",
                        f"go_b{layer}", f"if_b{layer}"]
                return [pp.tile([64, 2 * HH], F32, name=t, tag=t, bufs=1)
                        for t in tags]

            def half_post(ps_go, ps_if, CC, htag, h_half):
                """one half: psums -> h_half [64, 256] sbuf write."""
                nc.scalar.activation(CC[:, 0:HH], ps_go[:, 0:HH], Act.Tanh)
                sx = wp.tile([64, 2 * HH], F32, name=f"sx{htag}",
                             tag=f"sx{htag}")
                nc.scalar.activation(sx[:, :], ps_if[:, :], Act.Sigmoid)
                m_t = wp.tile([64, 2 * HH], F32, name=f"m{htag}",
                              tag=f"m{htag}")
                nc.vector.tensor_mul(m_t[:, :], sx[:, :], CC[:, :])
                nc.vector.tensor_add(CC[:, HH:2 * HH], m_t[:, 0:HH],
                                     m_t[:, HH:2 * HH])
                os_t = wp.tile([64, HH], F32, name=f"os{htag}",
                               tag=f"os{htag}")
                nc.scalar.activation(os_t[:, :], ps_go[:, HH:2 * HH],
                                     Act.Sigmoid)
                tc_s = wp.tile([64, HH], F32, name=f"tc{htag}",
                               tag=f"tc{htag}")
                nc.scalar.activation(tc_s[:, :], CC[:, HH:2 * HH], Act.Tanh)
                nc.vector.tensor_mul(h_half[:, :], os_t[:, :], tc_s[:, :])

            def transpose_half(h_half, trp, base_kk, htag):
                """h half [64,256] -> 2 hT chunk tiles [128,64]."""
                chunks = []
                for kk in range(2):
                    nc.tensor.transpose(trp[:, ts(kk, 64)],
                                        h_half[:, ts(kk, 128)], ident[:, :])
                for kk in range(2):
                    hTk = wp.tile([128, 64], F32R,
                                  name=f"hT{htag}k{base_kk + kk}",
                                  tag=f"hT{htag}k{base_kk + kk}", bufs=2)
                    nc.vector.tensor_copy(hTk[:, :], trp[:, ts(kk, 64)])
                    chunks.append(hTk)
                return chunks

            h1T = None
            h2T_prev = None
            ps1 = None      # [goa, ifa, gob, ifb] accumulating L1 gates for t

            def xy_stops(t, ps, start):
                xs = xyf[0:66, ts(t, 64)]
                for j in range(4):
                    nc.tensor.matmul(ps[j][:, :], xs, w1c0[:, ts(j, 512)],
                                     start=start, stop=True,
                                     skip_group_check=True)

            xy_done_for = -1
            for t in range(tp):
                first = t == 0
                # --- L2 groups open with bias matmuls (pure PE fill) ---
                ps2 = gate_tiles(2)
                for j in range(4):
                    nc.tensor.matmul(ps2[j][:, :], ONES, b2row[:, ts(j, 512)],
                                     start=True, stop=False,
                                     skip_group_check=True)
                # --- L1(t): finish gates with the xy chunk (AR: needs m) ---
                if ps1 is None:
                    ps1 = gate_tiles(1)
                if xy_done_for < t:
                    xy_stops(t, ps1, first)
                    xy_done_for = t
                # --- L2 h2-part (needs h2T(t-1)); covers the L1 post chain ---
                if not first:
                    for k in range(4):
                        wk = (4 + k) * G
                        st = h2T_prev[k][:, :]
                        for j in range(4):
                            nc.tensor.matmul(
                                ps2[j][:, :], st,
                                w2[:, wk + j * 512:wk + (j + 1) * 512],
                                start=False, stop=False,
                                skip_group_check=True)
                # --- L1 post, half-pipelined with transposes ---
                h1a = wp.tile([64, HH], F32, name="h1a", tag="h1a")
                h1b = wp.tile([64, HH], F32, name="h1b", tag="h1b")
                half_post(ps1[0], ps1[1], CC1[0], "1a", h1a)
                half_post(ps1[2], ps1[3], CC1[1], "1b", h1b)
                trp1a = pp.tile([128, 128], F32, name="trp1a",
                                tag="go_a1", bufs=1)
                c01 = transpose_half(h1a, trp1a, 0, "1")
                # --- L2 h1-part k=0 between the half transposes ---
                def l2h1_chunk(k, st):
                    wk = k * G
                    for j in range(4):
                        nc.tensor.matmul(
                            ps2[j][:, :], st,
                            w2[:, wk + j * 512:wk + (j + 1) * 512],
                            start=False, stop=(k == 3),
                            skip_group_check=True)
                l2h1_chunk(0, c01[0][:, :])
                trp1b = pp.tile([128, 128], F32, name="trp1b",
                                tag="go_b1", bufs=1)
                c23 = transpose_half(h1b, trp1b, 2, "1")
                h1T = c01 + c23
                for k in range(1, 4):
                    l2h1_chunk(k, h1T[k][:, :])
                # --- L1(t+1) h-part (pipelined ahead) ---
                if t < tp - 1:
                    nps1 = gate_tiles(1)
                    for k in range(4):
                        wk = k * G
                        st = h1T[k][:, :]
                        for j in range(4):
                            nc.tensor.matmul(
                                nps1[j][:, :], st,
                                w1h[:, wk + j * 512:wk + (j + 1) * 512],
                                start=(k == 0), stop=False,
                                skip_group_check=True)
                    ps1 = nps1
                    # conditioning: next xy chunk is static -> close the
                    # group now so L1 post(t+1) can start at step boundary
                    if t + 1 < to:
                        xy_stops(t + 1, ps1, False)
                        xy_done_for = t + 1
                else:
                    ps1 = None
                # --- L2 post + h2 transposes + head ---
                h2a = wp.tile([64, HH], F32, name="h2a", tag="h2a")
                h2b = wp.tile([64, HH], F32, name="h2b", tag="h2b")
                half_post(ps2[0], ps2[1], CC2[0], "2a", h2a)
                half_post(ps2[2], ps2[3], CC2[1], "2b", h2b)
                trp2a = pp.tile([128, 128], F32, name="trp2a",
                                tag="go_a2", bufs=1)
                d01 = transpose_half(h2a, trp2a, 0, "2")
                do_head = t >= to - 1
                if do_head:
                    mdp = pp.tile([64, 64], F32, name="mdp", tag="if_b2",
                                  bufs=1)
                    for k in range(2):
                        nc.tensor.matmul(mdp[:, :], wmd[:, ts(k, 64)],
                                         d01[k][:, :], start=(k == 0),
                                         stop=False, skip_group_check=True)
                trp2b = pp.tile([128, 128], F32, name="trp2b",
                                tag="go_b2", bufs=1)
                d23 = transpose_half(h2b, trp2b, 2, "2")
                h2T = d01 + d23
                if do_head:
                    for k in range(2, 4):
                        nc.tensor.matmul(mdp[:, :], wmd[:, ts(k, 64)],
                                         h2T[k][:, :], start=False,
                                         stop=(k == 3),
                                         skip_group_check=True)
                    # m -> feature row 0, slot t+1 (f32r rounding on write)
                    nc.scalar.activation(xyf[0:1, ts(t + 1, 64)],
                                         mdp[0:1, :], Act.Identity,
                                         bias=bmd[0:1, 0:1], scale=1.0)
                    if t >= to:
                        # d -> row 64 (ones/d row), slot t (already consumed)
                        nc.scalar.activation(xyf[64:65, ts(t, 64)],
                                             mdp[32:33, :], Act.Identity,
                                             bias=bmd[32:33, 0:1], scale=1.0)
                h2T_prev = h2T

            # ---- outputs: mean row = slots TO+1..TP, disp row = slots TO..TP-1
            nc.sync.dma_start(out_d[0:1, :],
                              xyf[0:1, (to + 1) * B:(tp + 1) * B].bitcast(F32))
            nc.sync.dma_start(out_d[1:2, :],
                              xyf[64:65, to * B:tp * B].bitcast(F32))

    n = split_excess_waits(nc) if split_waits else 0
    return nc, n


_CACHE = {}


def _get_program():
    if "nc" not in _CACHE:
        _CACHE["nc"] = build_program()[0]
    return _CACHE["nc"]


def make_core_inputs(x, y, W1, b1, W2, b2, Wm, bm, Wd, bd, tp=TP, to=TO):
    """Host-side prep: returns (in_maps list of 8 dicts, scale [512])."""
    NSLOT_ = tp + 1
    XCOLS_ = NSLOT_ * B
    x = np.asarray(x, np.float32)
    y = np.asarray(y, np.float32)
    W1 = np.asarray(W1, np.float32)
    b1 = np.asarray(b1, np.float32)
    W2 = np.asarray(W2, np.float32)
    b2 = np.asarray(b2, np.float32)
    Wm = np.asarray(Wm, np.float32)
    bm = np.asarray(bm, np.float32)
    Wd = np.asarray(Wd, np.float32)
    bd = np.asarray(bd, np.float32)

    scale = 1.0 + np.mean(y[:, 0:to, 0], axis=1)       # [512]
    y_sc = y[:, 0:to, 0] / scale[:, None]              # [512, to]

    b1a = b1.copy()
    b1a[2 * H:3 * H] += 1.0                             # forget-gate +1
    b2a = b2.copy()
    b2a[2 * H:3 * H] += 1.0

    P = gate_perm()
    W1p = W1[:, P]
    W2p = W2[:, P]
    b1p = b1a[P]
    b2p = b2a[P]

    # row layout: 0 = y/m, 1:64 = x[0:63], 64 = ones/bias (disp storage),
    # 65 = x[63]  (rows 0 and 64 must sit at legal engine partition bases)
    w1c0 = np.empty((66, G), np.float32)
    w1c0[0] = W1p[F]                                    # y/m weight row
    w1c0[1:64] = W1p[0:F - 1]                           # x weight rows 0..62
    w1c0[64] = b1p                                      # bias row (ones input)
    w1c0[65] = W1p[F - 1]                               # x weight row 63

    w1h = np.ascontiguousarray(
        W1p[F + 1:].reshape(4, 128, G).transpose(1, 0, 2).reshape(128, 4 * G))
    w2 = np.ascontiguousarray(
        W2p.reshape(8, 128, G).transpose(1, 0, 2).reshape(128, 8 * G))

    wmd = np.zeros((128, 4, 64), np.float32)
    wmd[:, :, 0] = Wm[:, 0].reshape(4, 128).T
    wmd[:, :, 32] = Wd[:, 0].reshape(4, 128).T
    wmd = np.ascontiguousarray(wmd.reshape(128, 4 * 64))

    b2row = np.ascontiguousarray(b2p.reshape(1, G))
    bmd = np.zeros((33, 1), np.float32)
    bmd[0, 0] = bm[0]
    bmd[32, 0] = bd[0]
    identity = np.eye(64, dtype=np.float32)

    in_maps = []
    for c in range(NC):
        bs = slice(c * B, (c + 1) * B)
        xyf = np.zeros((66, NSLOT_, B), np.float32)
        xyf[0, 1:to, :] = y_sc[bs, 0:to - 1].T          # shifted y feed
        xt = x[bs].transpose(2, 1, 0)                   # [f, t, b]
        xyf[1:64, 0:tp, :] = xt[0:F - 1, 0:tp]          # x rows 0..62
        xyf[65, 0:tp, :] = xt[F - 1, 0:tp]              # x row 63
        xyf[64, :, :] = 1.0                             # ones / bias row
        in_maps.append({
            "xyf_d": np.ascontiguousarray(xyf.reshape(66, XCOLS_)),
            "w1c0_d": w1c0, "w1h_d": w1h, "w2_d": w2, "wmd_d": wmd,
            "b2row_d": b2row, "bmd_d": bmd, "id_d": identity,
        })
    return in_maps, scale


def postprocess(results, scale, tp=TP, to=TO):
    """results: list of 8 dicts with out_d [2, (tp-to)*64] -> [512, tp-to, 2]."""
    out = np.empty((B_FULL, tp - to, 2), np.float32)
    for c in range(NC):
        r = results[c]["out_d"]
        mean_tb = r[0].reshape(tp - to, B)              # [t, b]
        dpre_tb = r[1].reshape(tp - to, B)
        bs = slice(c * B, (c + 1) * B)
        sc = scale[bs]
        out[bs, :, 0] = (mean_tb * sc[None, :]).T
        disp = np.logaddexp(dpre_tb, 0.0)               # softplus
        out[bs, :, 1] = (disp * np.sqrt(sc)[None, :]).T
    return out


def kernel(x, y, W1, b1, W2, b2, Wm, bm, Wd, bd):
    in_maps, scale = make_core_inputs(x, y, W1, b1, W2, b2, Wm, bm, Wd, bd)
    nc = _get_program()
    res = bass_utils.run_bass_kernel_spmd(nc, in_maps, core_ids=list(range(NC)))
    return postprocess(res.results, scale)
